# revision 1
# baseline (speedup 1.0000x reference)
"""Trainium2 Bass kernel for CrossAttention (sparse_attention variant).

Reference computation (shapes hardcoded):
  x [2, 1024, 1024], context [2, 4, 1024, 1024], doc_similarities [2, 4]
  q = x @ Wq, kv = ctx @ Wkv (k|v), dots = q k^T / sqrt(d) + doc_bias,
  attn = softmax(dots over all 4096 doc tokens), out = (attn @ v) @ Wout + bout

Sharding: 8 cores = 2 batches x 4 head-pairs.  Core c: batch c//4, heads
{2*(c%4), 2*(c%4)+1}.  Each core computes a [1024, 1024] partial of the
output projection (its heads' rows of Wout); host sums 4 partials per batch.

On-core layout strategy (all matmuls f32r, full PE rate at free dim 512):
  qT [hd, n], kT [hd, j], vT [hd, j] from projections directly (contraction
  over d with weight tiles stationary, xT/ctxT moving).  S^T tiles [j, i]
  from QK with kT slices stationary.  Softmax along the PSUM partition axis
  (j): exp via ScalarE with fused scale + per-partition doc bias (no max
  subtraction needed; logits are O(5)); denominator via ones-vector matmul;
  V-natural tiles produced on the fly by PE transpose of vT slices
  (software-pipelined one j ahead); EV with V tiles stationary producing
  unnormalized Y^T [hd, i]; normalize by PE-broadcast reciprocal; partial
  output projection with normalized Y^T slices stationary.
"""

import numpy as np
from contextlib import ExitStack

import concourse.bass as bass
import concourse.mybir as mybir
import concourse.tile as tile
from concourse import bacc
from concourse import bass_utils
from concourse.masks import make_identity

# Problem constants
B, N, M, CN, D = 2, 1024, 4, 1024, 1024
H = 8          # total heads
HPC = 2        # heads per core
NCORES = 8
HD = D // H    # 128
J = M * CN     # 4096
KT = D // 128  # 8 contraction k-tiles
IC = N // 512  # 2 i-chunks of queries
JC = J // 512  # 8 j-chunks (projection granularity)
JT = J // 128  # 32 j-tiles (attention granularity)
SCALE = float(D ** -0.5)

FR = mybir.dt.float32r
F32 = mybir.dt.float32

_NC_CACHE = {}
LAST_RESULT = None


def _build_module(reps=1):
    nc = bacc.Bacc(
        "TRN2",
        target_bir_lowering=False,
        debug=False,
        num_devices=NCORES,
    )

    xT = nc.dram_tensor("xT", [D, N], FR, kind="ExternalInput").ap()
    ctxT = nc.dram_tensor("ctxT", [D, J], FR, kind="ExternalInput").ap()
    wq = nc.dram_tensor("wq", [128, KT * HPC * HD], FR, kind="ExternalInput").ap()
    wk = nc.dram_tensor("wk", [128, KT * HPC * HD], FR, kind="ExternalInput").ap()
    wv = nc.dram_tensor("wv", [128, KT * HPC * HD], FR, kind="ExternalInput").ap()
    wout = nc.dram_tensor("wout", [128, HPC * D], FR, kind="ExternalInput").ap()
    docb = nc.dram_tensor("docb", [128, JT], F32, kind="ExternalInput").ap()
    outp = nc.dram_tensor("outp", [N, D], F32, kind="ExternalOutput").ap()

    EXP = mybir.ActivationFunctionType.Exp

    with tile.TileContext(nc) as tc:
        with ExitStack() as ctx:
          wpool = ctx.enter_context(tc.tile_pool(name="wpool", bufs=1))
          big = ctx.enter_context(tc.tile_pool(name="big", bufs=1))
          stream = ctx.enter_context(tc.tile_pool(name="stream", bufs=4))
          epool = ctx.enter_context(tc.tile_pool(name="epool", bufs=4))
          vnpool = ctx.enter_context(tc.tile_pool(name="vnpool", bufs=4))
          spool = ctx.enter_context(tc.tile_pool(name="spool", bufs=2))
          pp = ctx.enter_context(tc.tile_pool(name="pp", bufs=8, space="PSUM"))
          for _rep in range(reps):

              docb_sb = wpool.tile([128, JT], F32, name="docb_sb")
              # memset/affine_select emit invalid ISA for float32r directly;
              # build the constants in f32 and convert via tensor_copy.
              ones_col_f = wpool.tile([128, 1], F32, name="ones_col_f")
              nc.vector.memset(ones_col_f[:, :], 1.0)
              ones_col = wpool.tile([128, 1], FR, name="ones_col")
              nc.vector.tensor_copy(ones_col[:, :], ones_col_f[:, :])
              ones_row_f = wpool.tile([1, 128], F32, name="ones_row_f")
              nc.vector.memset(ones_row_f[:, :], 1.0)
              ones_row = wpool.tile([1, 128], FR, name="ones_row")
              nc.vector.tensor_copy(ones_row[:, :], ones_row_f[:, :])
              ident_f = wpool.tile([128, 128], F32, name="ident_f")
              make_identity(nc, ident_f[:, :])
              ident = wpool.tile([128, 128], FR, name="ident")
              nc.vector.tensor_copy(ident[:, :], ident_f[:, :])

              # weight tiles (DMAs interleaved at first use below)
              wq_sb = wpool.tile([128, KT, HPC * HD], FR, name="wq_sb")
              wk_sb = wpool.tile([128, KT, HPC * HD], FR, name="wk_sb")
              wv_sb = wpool.tile([128, KT, HPC * HD], FR, name="wv_sb")
              wout_sb = wpool.tile([128, HPC, D], FR, name="wout_sb")

              # persistent per-head activations
              qT_sb = big.tile([128, HPC, N], FR, name="qT_sb")    # q^T  [hd, h, i]
              kT_sb = big.tile([128, HPC, J], FR, name="kT_sb")    # k^T  [hd, h, j]
              vT_sb = big.tile([128, HPC, J], FR, name="vT_sb")    # v^T  [hd, h, j]
              yn_sb = big.tile([128, HPC, N], FR, name="yn_sb")    # Ynorm^T [hd, h, i]

              # ---- interleaved Q/K/V projections ----
              # KV(jc) is the bulk; one 2-ktile slice of the Q projection rides
              # along each jc iteration so its DMA spreads across the phase.
              qp = {}
              for s in range(JC):
                  jc = s
                  q_ic = s // 4
                  q_kts = (2 * (s % 4), 2 * (s % 4) + 1)
                  # KV(jc)
                  kp = [
                      pp.tile([128, 512], F32, name=f"kp{h}_{jc}", tag="pp")
                      for h in range(HPC)
                  ]
                  vp = [
                      pp.tile([128, 512], F32, name=f"vp{h}_{jc}", tag="pp")
                      for h in range(HPC)
                  ]
                  for kt in range(KT):
                      if s == 0 and kt % 2 == 0:
                          c = kt // 2
                          nc.sync.dma_start(
                              out=wk_sb[:, 2 * c:2 * c + 2, :],
                              in_=wk[:, c * 512:(c + 1) * 512],
                          )
                          nc.sync.dma_start(
                              out=wv_sb[:, 2 * c:2 * c + 2, :],
                              in_=wv[:, c * 512:(c + 1) * 512],
                          )
                      ct = stream.tile([128, 512], FR, name="ct", tag="ct", bufs=10)
                      nc.sync.dma_start(
                          out=ct[:, :],
                          in_=ctxT[kt * 128:(kt + 1) * 128, jc * 512:(jc + 1) * 512],
                      )
                      if s < 4 and kt == 1:
                          nc.sync.dma_start(
                              out=wq_sb[:, 2 * s:2 * s + 2, :],
                              in_=wq[:, s * 512:(s + 1) * 512],
                          )
                      if s == 0 and kt == 1:
                          nc.sync.dma_start(out=docb_sb[:, :], in_=docb[:, :])
                      for h in range(HPC):
                          nc.tensor.matmul(
                              kp[h][:, :],
                              lhsT=wk_sb[:, kt, h * HD:(h + 1) * HD],
                              rhs=ct[:, :],
                              start=(kt == 0),
                              stop=(kt == KT - 1),
                          )
                          nc.tensor.matmul(
                              vp[h][:, :],
                              lhsT=wv_sb[:, kt, h * HD:(h + 1) * HD],
                              rhs=ct[:, :],
                              start=(kt == 0),
                              stop=(kt == KT - 1),
                          )
                  # Q slice
                  if s % 4 == 0:
                      qp[q_ic] = [
                          pp.tile([128, 512], F32, name=f"qp{h}_{q_ic}", tag="pp")
                          for h in range(HPC)
                      ]
                  for kt in q_kts:
                      xt = stream.tile([128, 512], FR, name="xt", tag="xt", bufs=6)
                      nc.sync.dma_start(
                          out=xt[:, :],
                          in_=xT[kt * 128:(kt + 1) * 128, q_ic * 512:(q_ic + 1) * 512],
                      )
                      for h in range(HPC):
                          nc.tensor.matmul(
                              qp[q_ic][h][:, :],
                              lhsT=wq_sb[:, kt, h * HD:(h + 1) * HD],
                              rhs=xt[:, :],
                              start=(kt == 0),
                              stop=(kt == KT - 1),
                          )
                  for h in range(HPC):
                      nc.vector.tensor_copy(
                          kT_sb[:, h, jc * 512:(jc + 1) * 512], kp[h][:, :]
                      )
                      if s == JC - 1:
                          # final slice: ScalarE is idle until the first exp;
                          # split the eviction backlog across both engines
                          nc.scalar.copy(
                              vT_sb[:, h, jc * 512:(jc + 1) * 512], vp[h][:, :]
                          )
                      else:
                          nc.vector.tensor_copy(
                              vT_sb[:, h, jc * 512:(jc + 1) * 512], vp[h][:, :]
                          )
                  if s % 4 == 3:
                      for h in range(HPC):
                          nc.vector.tensor_copy(
                              qT_sb[:, h, q_ic * 512:(q_ic + 1) * 512],
                              qp[q_ic][h][:, :],
                          )
                      del qp[q_ic]
                  if jc == 0:
                      # out-projection weights: load during the KV phase
                      nc.sync.dma_start(out=wout_sb[:, :, :], in_=wout[:, :])

              # ---- attention, one head at a time ----
              # Each head: QK -> exp (fused scale+bias) -> EV + denominator,
              # software-pipelined one j-tile ahead.  The normalization
              # epilogue of head h is emitted after head h+1's prologue so PE
              # keeps streaming matmuls across the head boundary.
              pending_epilogue = None
              for h in range(HPC):
                  st_tiles = {}
                  vn_tiles = {}

                  def emit_qk(j, h=h, st_tiles=st_tiles):
                      for ic in range(IC):
                          st = pp.tile([128, 512], F32, name=f"st{h}", tag="pp")
                          nc.tensor.matmul(
                              st[:, :],
                              lhsT=kT_sb[:, h, j * 128:(j + 1) * 128],
                              rhs=qT_sb[:, h, ic * 512:(ic + 1) * 512],
                              start=True,
                              stop=True,
                          )
                          st_tiles[(j, ic)] = st

                  def emit_vtrans(j, h=h, vn_tiles=vn_tiles):
                      # V natural tile [j, hd] <- PE transpose of vT slice
                      tp = pp.tile([128, 128], FR, name=f"tp{h}", tag="pp")
                      nc.tensor.transpose(
                          tp[:, :], vT_sb[:, h, j * 128:(j + 1) * 128], ident[:, :]
                      )
                      vn = vnpool.tile([128, 128], FR, name=f"vn{h}", tag="vn")
                      nc.vector.tensor_copy(vn[:, :], tp[:, :])
                      vn_tiles[j] = vn

                  emit_qk(0)
                  emit_vtrans(0)
                  if pending_epilogue is not None:
                      pending_epilogue()
                      pending_epilogue = None
                  y = [
                      pp.tile([128, 512], F32, name=f"y{h}_{ic}", tag="pp")
                      for ic in range(IC)
                  ]
                  dn = [
                      pp.tile([1, 512], F32, name=f"dn{h}_{ic}", tag="pp")
                      for ic in range(IC)
                  ]
                  for j in range(JT):
                      if j + 1 < JT:
                          emit_qk(j + 1)
                          emit_vtrans(j + 1)
                      et = epool.tile([128, N], FR, name=f"et{h}", tag="et")
                      for ic in range(IC):
                          st = st_tiles.pop((j, ic))
                          nc.scalar.activation(
                              et[:, ic * 512:(ic + 1) * 512],
                              st[:, :],
                              EXP,
                              bias=docb_sb[:, j:j + 1],
                              scale=SCALE,
                          )
                      vn = vn_tiles.pop(j)
                      for ic in range(IC):
                          nc.tensor.matmul(
                              y[ic][:, :],
                              lhsT=vn[:, :],
                              rhs=et[:, ic * 512:(ic + 1) * 512],
                              start=(j == 0),
                              stop=(j == JT - 1),
                          )
                          nc.tensor.matmul(
                              dn[ic][:, :],
                              lhsT=ones_col[:, :],
                              rhs=et[:, ic * 512:(ic + 1) * 512],
                              start=(j == 0),
                              stop=(j == JT - 1),
                          )

                  def epilogue(h=h, y=y, dn=dn):
                      # normalize: yn^T = y^T * broadcast(1/denominator)
                      recip = spool.tile([1, N], FR, name=f"recip{h}", tag="recip")
                      for ic in range(IC):
                          with nc.allow_low_precision(
                              reason="float32r output is 32-bit, same as float32"
                          ):
                              nc.vector.reciprocal(
                                  recip[:, ic * 512:(ic + 1) * 512], dn[ic][:, :]
                              )
                      rs = spool.tile([128, N], FR, name=f"rs{h}", tag="rs")
                      for ic in range(IC):
                          bc = pp.tile([128, 512], F32, name=f"bc{h}", tag="pp")
                          nc.tensor.matmul(
                              bc[:, :],
                              lhsT=ones_row[:, :],
                              rhs=recip[:, ic * 512:(ic + 1) * 512],
                              start=True,
                              stop=True,
                          )
                          nc.scalar.copy(rs[:, ic * 512:(ic + 1) * 512], bc[:, :])
                          nc.vector.tensor_mul(
                              yn_sb[:, h, ic * 512:(ic + 1) * 512],
                              y[ic][:, :],
                              rs[:, ic * 512:(ic + 1) * 512],
                          )

                  pending_epilogue = epilogue
              pending_epilogue()

              # ---- partial output projection ----
              for it in range(N // 128):
                  for oc in range(D // 512):
                      op = pp.tile([128, 512], F32, name="op", tag="pp")
                      for h in range(HPC):
                          nc.tensor.matmul(
                              op[:, :],
                              lhsT=yn_sb[:, h, it * 128:(it + 1) * 128],
                              rhs=wout_sb[:, h, oc * 512:(oc + 1) * 512],
                              start=(h == 0),
                              stop=(h == HPC - 1),
                          )
                      ot = stream.tile([128, 512], F32, name="ot", tag="ot")
                      nc.vector.tensor_copy(ot[:, :], op[:, :])
                      nc.sync.dma_start(
                          out=outp[it * 128:(it + 1) * 128, oc * 512:(oc + 1) * 512],
                          in_=ot[:, :],
                      )

    nc.compile()
    return nc


def get_nc(reps=1):
    if reps not in _NC_CACHE:
        _NC_CACHE[reps] = _build_module(reps)
    return _NC_CACHE[reps]


def make_in_maps(inputs):
    x = np.asarray(inputs["x"], dtype=np.float32)
    context = np.asarray(inputs["context"], dtype=np.float32)
    doc = np.asarray(inputs["doc_similarities"], dtype=np.float32)
    cmask = np.asarray(inputs["context_mask"])
    Wq = np.asarray(inputs["Wq"], dtype=np.float32)
    Wkv = np.asarray(inputs["Wkv"], dtype=np.float32)
    beta = float(np.asarray(inputs["beta"]))
    Wout = np.asarray(inputs["Wout"], dtype=np.float32)

    per_batch = []
    for b in range(B):
        xT = np.ascontiguousarray(x[b].T)
        ctxT = np.ascontiguousarray(context[b].reshape(J, D).T)
        bias = np.repeat(doc[b], CN) * beta
        bias = np.where(cmask[b].reshape(J), bias, -1e30).astype(np.float32)
        docb = np.ascontiguousarray(bias.reshape(JT, 128).T)  # [128, JT]
        per_batch.append((xT, ctxT, docb))

    in_maps = []
    for c in range(NCORES):
        b = c // 4
        h0 = (c % 4) * HPC
        xT, ctxT, docb = per_batch[b]
        def pack_kxc(w):
            # [D, C] -> [128, KT*C]: tile rows so each partition line is contiguous
            c = w.shape[1]
            return np.ascontiguousarray(
                w.reshape(KT, 128, c).transpose(1, 0, 2).reshape(128, KT * c)
            )

        wout_c = Wout[h0 * HD:(h0 + HPC) * HD, :]
        in_maps.append({
            "xT": xT,
            "ctxT": ctxT,
            "wq": pack_kxc(Wq[:, h0 * HD:(h0 + HPC) * HD]),
            "wk": pack_kxc(Wkv[:, h0 * HD:(h0 + HPC) * HD]),
            "wv": pack_kxc(Wkv[:, D + h0 * HD:D + (h0 + HPC) * HD]),
            "wout": np.ascontiguousarray(
                wout_c.reshape(HPC, 128, D).transpose(1, 0, 2).reshape(128, HPC * D)
            ),
            "docb": docb,
        })
    return in_maps


def kernel(**inputs):
    global LAST_RESULT
    nc = get_nc()
    in_maps = make_in_maps(inputs)
    res = bass_utils.run_bass_kernel_spmd(
        nc, in_maps, core_ids=list(range(NCORES))
    )
    LAST_RESULT = res
    out = np.zeros((B, N, D), dtype=np.float32)
    for c in range(NCORES):
        out[c // 4] += res.results[c]["outp"]
    out += np.asarray(inputs["bout"], dtype=np.float32)
    return out

